# revision 4
# baseline (speedup 1.0000x reference)
"""3-layer GAT on 8 Trainium2 NeuronCores.

Strategy (dst-sharded):
- Core k owns destination nodes [6250k, 6250(k+1)).
- Host partitions edges by dst owner, groups them into 49 blocks of 128 dst
  nodes, pads each block's edge list to whole 128-edge tiles (pad edges gather
  row 0 and carry one-hot position 255 => contribute exactly zero).
- Per layer: each core computes its slice of feat/el/er with ONE matmul using
  extended weights [W | W@al | W@ar] (el/er fold into the projection), writes
  [feat|el] rows to a DRAM table slice, AllGathers the full table.
- Edge phase per 128-dst block: dma_gather pulls [feat|el] rows of edge
  sources (int16 indices, so the 50000-row table is split in two halves);
  a one-hot matrix oh[e,d] = (dstpos[e]==d) built by a compare against a
  constant iota matrix both scatters (PSUM-accumulating matmul of
  [ex*feat | ex] -> [unnorm | denom]) and, transposed via TensorE, expands
  er[dst] per edge. Softmax max-subtraction is dropped (scores are O(1); the
  softmax is shift-invariant).
"""
import numpy as np

N = 50000
E = 500000
NC = 8
NLOC = N // NC          # 6250
P = 128
NBT = 49                # node tiles / blocks per core (48*128 + 106)
LAST_ROWS = NLOC - 48 * P   # 106
HALF = 32768            # int16 index split
IN = 128
D = 256
H12 = 4
F = 64
CLS = 64
TW12 = 320              # table row f32 words (256 feat + 4 el + pad) -> 1280B
TW3 = 128               # (64 feat + 1 el + pad) -> 512B
SLOPE = 0.2


def _wrap_idx16(ix):
    """[128] int16 -> dma_gather wrapped layout [128, 8]."""
    return np.tile(ix.reshape(8, 16).T, (8, 1)).astype(np.int16)


def make_schedule(src, dst):
    """Uniform (across cores) tile schedule + per-core index/position data."""
    src = np.asarray(src).astype(np.int64)
    dst = np.asarray(dst).astype(np.int64)
    owner = dst // NLOC
    per_core = []
    cnt = np.zeros((NC, NBT, 2), np.int64)
    for k in range(NC):
        m = owner == k
        s = src[m]
        dl = dst[m] - k * NLOC
        blk = dl // P
        pos = dl % P
        half = (s >= HALF).astype(np.int64)
        order = np.lexsort((half, blk))
        per_core.append((s[order], blk[order], pos[order], half[order]))
        for b in range(NBT):
            mb = blk[order] == b
            hb = half[order][mb]
            cnt[k, b, 0] = int((hb == 0).sum())
            cnt[k, b, 1] = int((hb == 1).sum())

    TA = np.maximum(np.ceil(cnt[:, :, 0] / P).astype(int).max(axis=0), 0)
    TB = np.maximum(np.ceil(cnt[:, :, 1] / P).astype(int).max(axis=0), 0)
    tile_block = []
    tile_half = []
    for b in range(NBT):
        tile_block += [b] * (TA[b] + TB[b])
        tile_half += [0] * TA[b] + [1] * TB[b]
    TT = len(tile_block)

    idx16 = np.zeros((NC, P, TT * 8), np.int16)
    dstpos = np.full((NC, P, TT), 255.0, np.float32)
    t0 = 0
    for b in range(NBT):
        for k in range(NC):
            s, blk, pos, half = per_core[k]
            mb = blk == b
            sb, pb, hb = s[mb], pos[mb], half[mb]
            for hv, Tn, toff in ((0, TA[b], 0), (1, TB[b], TA[b])):
                sel = hb == hv
                ss = sb[sel] - hv * HALF
                pp = pb[sel]
                nfull = len(ss)
                buf_i = np.zeros(Tn * P, np.int16)
                buf_p = np.full(Tn * P, 255.0, np.float32)
                buf_i[:nfull] = ss.astype(np.int16)
                buf_p[:nfull] = pp.astype(np.float32)
                for j in range(Tn):
                    t = t0 + toff + j
                    idx16[k, :, t * 8:(t + 1) * 8] = _wrap_idx16(buf_i[j * P:(j + 1) * P])
                    dstpos[k, :, t] = buf_p[j * P:(j + 1) * P]
        t0 += TA[b] + TB[b]
    return tile_block, tile_half, TT, idx16, dstpos


def build_nc(tile_block, tile_half, TT, n_layers=3, dbg=None):
    import concourse.bacc as bacc
    import concourse.bass as bass
    import concourse.mybir as mybir
    import concourse.tile as tile
    from concourse.library_config import mlp
    dt = mybir.dt

    nc = bacc.Bacc("TRN2", target_bir_lowering=False, debug=False,
                   num_devices=NC, num_swdge_queues=4)

    xT = nc.declare_dram_parameter("xT", [IN, NBT * P], dt.float32, isOutput=False)
    w1 = nc.declare_dram_parameter("w1", [IN, D + 8], dt.float32, isOutput=False)
    w2 = nc.declare_dram_parameter("w2", [D, D + 8], dt.float32, isOutput=False)
    w3 = nc.declare_dram_parameter("w3", [D, CLS + 2 + CLS], dt.float32, isOutput=False)
    idx_in = nc.declare_dram_parameter("idx16", [P, TT * 8], dt.int16, isOutput=False)
    dpos_in = nc.declare_dram_parameter("dstpos", [P, TT], dt.float32, isOutput=False)
    iota_in = nc.declare_dram_parameter("iota", [P, P], dt.float32, isOutput=False)
    ident_in = nc.declare_dram_parameter("ident", [P, P], dt.float32, isOutput=False)
    outp = nc.declare_dram_parameter("out", [NLOC, CLS], dt.float32, isOutput=True)
    dbg_t = None
    if dbg is not None:
        dbg_t = nc.declare_dram_parameter("dbg", [NBT * P, TW12], dt.float32, isOutput=True)

    slice12 = nc.dram_tensor("slice12", [NLOC, TW12], dt.float32)
    table12 = nc.dram_tensor("table12", [N, TW12], dt.float32, addr_space="Shared")
    slice3 = nc.dram_tensor("slice3", [NLOC, TW3], dt.float32)
    table3 = nc.dram_tensor("table3", [N, TW3], dt.float32, addr_space="Shared")

    groups = [list(range(NC))]

    with tile.TileContext(nc) as tc:
        with (
            tc.tile_pool(name="pers", bufs=1) as pers,
            tc.tile_pool(name="kt", bufs=3) as ktp,
            tc.tile_pool(name="stage", bufs=3) as stp,
            tc.tile_pool(name="gblk", bufs=2) as gp,
            tc.tile_pool(name="ohblk", bufs=2) as ohp,
            tc.tile_pool(name="ohT", bufs=3) as ohtp,
            tc.tile_pool(name="small", bufs=3) as smp,
            tc.tile_pool(name="vals", bufs=2) as vp,
            tc.tile_pool(name="otile", bufs=2) as op_,
            tc.tile_pool(name="ps_feat", bufs=2, space="PSUM") as psf,
            tc.tile_pool(name="ps_out", bufs=2, space="PSUM") as pso,
            tc.tile_pool(name="ps_tr", bufs=2, space="PSUM") as pstr,
            tc.tile_pool(name="ps_er", bufs=2, space="PSUM") as pser,
        ):
            nc.gpsimd.load_library(mlp)
            # persistent SBUF state
            xT_sb = pers.tile([P, NBT * P], dt.float32)
            nc.sync.dma_start(xT_sb[:], xT[:])
            w1_sb = pers.tile([P, D + 8], dt.float32)
            nc.sync.dma_start(w1_sb[:], w1[:])
            w2_sb = pers.tile([P, 2 * (D + 8)], dt.float32)
            w3_sb = pers.tile([P, 2 * (CLS + 2 + CLS)], dt.float32)
            for kt in range(2):
                nc.sync.dma_start(w2_sb[:, kt * (D + 8):(kt + 1) * (D + 8)],
                                  w2[kt * P:(kt + 1) * P, :])
                nc.sync.dma_start(w3_sb[:, kt * (CLS + 2 + CLS):(kt + 1) * (CLS + 2 + CLS)],
                                  w3[kt * P:(kt + 1) * P, :])
            idx_sb = pers.tile([P, TT * 8], dt.int16)
            nc.sync.dma_start(idx_sb[:], idx_in[:])
            dpos_sb = pers.tile([P, TT], dt.float32)
            nc.sync.dma_start(dpos_sb[:], dpos_in[:])
            iota_sb = pers.tile([P, P], dt.float32)
            nc.sync.dma_start(iota_sb[:], iota_in[:])
            ident_sb = pers.tile([P, P], dt.float32)
            nc.sync.dma_start(ident_sb[:], ident_in[:])
            h_sb = pers.tile([P, NBT * D], dt.float32)
            er_sb = pers.tile([P, NBT * H12], dt.float32)
            er3_sb = pers.tile([P, NBT], dt.float32)
            res_sb = pers.tile([P, NBT * CLS], dt.float32)

            # per-block tile ranges
            blocks = []
            t0 = 0
            for b in range(NBT):
                ts = [t for t in range(len(tile_block)) if tile_block[t] == b]
                blocks.append(ts)

            tabA12 = table12[0:HALF, :]
            tabB12 = table12[HALF:N, :]
            tabA3 = table3[0:HALF, :]
            tabB3 = table3[HALF:N, :]

            qn = [0]

            def edge_phase(layer):
                if layer < 2:
                    TW, FO, NH, tabA, tabB = TW12, D, H12, tabA12, tabB12
                    er_l = er_sb
                else:
                    TW, FO, NH, tabA, tabB = TW3, CLS, 1, tabA3, tabB3
                    er_l = er3_sb
                W2c = FO + NH          # vals row width
                for b in range(NBT):
                    ts = blocks[b]
                    T = len(ts)
                    Gblk = gp.tile([P, T * TW], dt.float32, tag="G")
                    ohblk = ohp.tile([P, T * P], dt.float32, tag="oh")
                    er_ps = pser.tile([P, T * NH], dt.float32, tag="erp")
                    for j, t in enumerate(ts):
                        nc.gpsimd.dma_gather(
                            Gblk[:, j * TW:(j + 1) * TW].rearrange("p (c e) -> p c e", c=1),
                            tabA if tile_half[t] == 0 else tabB,
                            idx_sb[:, t * 8:(t + 1) * 8],
                            P, P, TW, queue_num=qn[0] % 4,
                        )
                        qn[0] += 1
                        nc.vector.tensor_scalar(
                            out=ohblk[:, j * P:(j + 1) * P],
                            in0=iota_sb[:],
                            scalar1=dpos_sb[:, t:t + 1],
                            scalar2=None,
                            op0=mybir.AluOpType.is_equal,
                        )
                        ohT_ps = pstr.tile([P, P], dt.float32, tag="trp")
                        nc.tensor.transpose(ohT_ps[:], ohblk[:, j * P:(j + 1) * P], ident_sb[:])
                        ohT = ohtp.tile([P, P], dt.float32, tag="ohT")
                        nc.vector.tensor_copy(ohT[:], ohT_ps[:])
                        nc.tensor.matmul(er_ps[:, j * NH:(j + 1) * NH], ohT[:],
                                         er_l[:, b * NH:(b + 1) * NH], start=True, stop=True)
                    e_sb = smp.tile([P, T * NH], dt.float32, tag="e")
                    nc.vector.tensor_tensor(
                        out=e_sb[:],
                        in0=Gblk[:].rearrange("p (t c) -> p t c", t=T)[:, :, FO:FO + NH],
                        in1=er_ps[:, :T * NH],
                        op=mybir.AluOpType.add,
                    )
                    es_sb = smp.tile([P, T * NH], dt.float32, tag="es")
                    nc.vector.tensor_scalar_mul(es_sb[:], e_sb[:], SLOPE)
                    nc.vector.tensor_tensor(out=e_sb[:], in0=e_sb[:], in1=es_sb[:],
                                            op=mybir.AluOpType.max)
                    ex_sb = smp.tile([P, T * NH], dt.float32, tag="ex")
                    nc.scalar.activation(ex_sb[:], e_sb[:], mybir.ActivationFunctionType.Exp)
                    vals = vp.tile([P, T * W2c], dt.float32, tag="v")
                    nc.vector.tensor_tensor(
                        out=vals[:].rearrange("p (t c) -> p t c", t=T)[:, :, 0:FO]
                            .rearrange("p t (h f) -> p t h f", h=NH),
                        in0=Gblk[:].rearrange("p (t c) -> p t c", t=T)[:, :, 0:FO]
                            .rearrange("p t (h f) -> p t h f", h=NH),
                        in1=ex_sb[:].rearrange("p (t h) -> p t h", t=T)
                            .to_broadcast([P, T, NH, F]),
                        op=mybir.AluOpType.mult,
                    )
                    nc.vector.tensor_copy(
                        vals[:].rearrange("p (t c) -> p t c", t=T)[:, :, FO:FO + NH],
                        ex_sb[:].rearrange("p (t h) -> p t h", t=T),
                    )
                    out_ps = pso.tile([P, W2c], dt.float32, tag="outp")
                    for j in range(T):
                        nc.tensor.matmul(out_ps[:], ohblk[:, j * P:(j + 1) * P],
                                         vals[:, j * W2c:(j + 1) * W2c],
                                         start=(j == 0), stop=(j == T - 1))
                    den = smp.tile([P, NH], dt.float32, tag="den")
                    nc.vector.tensor_scalar_max(den[:], out_ps[:, FO:FO + NH], 1e-30)
                    rec = smp.tile([P, NH], dt.float32, tag="rec")
                    nc.vector.reciprocal(rec[:], den[:])
                    o_t = op_.tile([P, FO], dt.float32, tag="ot")
                    nc.vector.tensor_tensor(
                        out=o_t[:].rearrange("p (h f) -> p h f", h=NH),
                        in0=out_ps[:, 0:FO].rearrange("p (h f) -> p h f", h=NH),
                        in1=rec[:].to_broadcast([P, NH, F]),
                        op=mybir.AluOpType.mult,
                    )
                    # layer tails
                    if layer == 0:
                        _elu_into(o_t, h_sb, b, D)
                    elif layer == 1:
                        pre = op_.tile([P, D], dt.float32, tag="pre")
                        nc.vector.tensor_tensor(out=pre[:], in0=o_t[:],
                                                in1=h_sb[:, b * D:(b + 1) * D],
                                                op=mybir.AluOpType.add)
                        _elu_into(pre, h_sb, b, D)
                    else:
                        lg = op_.tile([P, CLS], dt.float32, tag="lg")
                        nc.vector.tensor_tensor(out=lg[:], in0=o_t[:],
                                                in1=res_sb[:, b * CLS:(b + 1) * CLS],
                                                op=mybir.AluOpType.add)
                        rows = P if b < NBT - 1 else LAST_ROWS
                        if dbg != "noout":
                            nc.sync.dma_start(outp[b * P:b * P + rows, :], lg[0:rows, :])
                        else:
                            nc.vector.tensor_copy(h_sb[:, b * CLS:(b + 1) * CLS], lg[:])

            def _elu_into(x_t, dst_sb, b, width):
                # elu(x) = max(x, exp(min(x,0)) - 1)
                t1 = op_.tile([P, width], dt.float32, tag="elu1")
                nc.vector.tensor_scalar_min(t1[:], x_t[:], 0.0)
                nc.scalar.activation(t1[:], t1[:], mybir.ActivationFunctionType.Exp)
                nc.vector.tensor_scalar_add(t1[:], t1[:], -1.0)
                nc.vector.tensor_tensor(out=dst_sb[:, b * width:(b + 1) * width],
                                        in0=x_t[:], in1=t1[:], op=mybir.AluOpType.max)

            def feat_phase(layer):
                if layer == 0:
                    wsb, wcols, nk = w1_sb, D + 8, 1
                elif layer == 1:
                    wsb, wcols, nk = w2_sb, D + 8, 2
                else:
                    wsb, wcols, nk = w3_sb, CLS + 2 + CLS, 2
                for nt in range(NBT):
                    f_ps = psf.tile([P, wcols], dt.float32, tag="fp")
                    for kt in range(nk):
                        if layer == 0:
                            lhsT = xT_sb[:, nt * P:(nt + 1) * P]
                        else:
                            tr_ps = pstr.tile([P, P], dt.float32, tag="trp")
                            nc.tensor.transpose(
                                tr_ps[:], h_sb[:, nt * D + kt * P: nt * D + (kt + 1) * P],
                                ident_sb[:])
                            ktile = ktp.tile([P, P], dt.float32, tag="kt")
                            nc.vector.tensor_copy(ktile[:], tr_ps[:])
                            lhsT = ktile[:]
                        nc.tensor.matmul(f_ps[:], lhsT, wsb[:, kt * wcols:(kt + 1) * wcols],
                                         start=(kt == 0), stop=(kt == nk - 1))
                    rows = P if nt < NBT - 1 else LAST_ROWS
                    if layer < 2:
                        st = stp.tile([P, D + H12], dt.float32, tag="st")
                        nc.vector.tensor_copy(st[:], f_ps[:, 0:D + H12])
                        nc.vector.tensor_copy(er_sb[:, nt * H12:(nt + 1) * H12],
                                              f_ps[:, D + H12:D + 2 * H12])
                        nc.sync.dma_start(slice12[nt * P:nt * P + rows, 0:D + H12],
                                          st[0:rows, :])
                        if dbg == "feat":
                            nc.sync.dma_start(dbg_t[nt * P:nt * P + rows, 0:D + H12],
                                              st[0:rows, :])
                    else:
                        st = stp.tile([P, CLS + 1], dt.float32, tag="st3")
                        nc.vector.tensor_copy(st[:], f_ps[:, 0:CLS + 1])
                        nc.vector.tensor_copy(er3_sb[:, nt:nt + 1],
                                              f_ps[:, CLS + 1:CLS + 2])
                        nc.vector.tensor_copy(res_sb[:, nt * CLS:(nt + 1) * CLS],
                                              f_ps[:, CLS + 2:CLS + 2 + CLS])
                        nc.sync.dma_start(slice3[nt * P:nt * P + rows, 0:CLS + 1],
                                          st[0:rows, :])

            for layer in range(n_layers):
                feat_phase(layer)
                if dbg == "feat" and layer == n_layers - 1:
                    break
                if layer < 2:
                    nc.gpsimd.collective_compute(
                        "AllGather", mybir.AluOpType.bypass, replica_groups=groups,
                        ins=[slice12[:, :]], outs=[table12[:, :]])
                else:
                    nc.gpsimd.collective_compute(
                        "AllGather", mybir.AluOpType.bypass, replica_groups=groups,
                        ins=[slice3[:, :]], outs=[table3[:, :]])
                if dbg == "table" and layer == n_layers - 1:
                    for nt in range(NBT):
                        gtb = stp.tile([P, D + H12], dt.float32, tag="st")
                        nc.sync.dma_start(gtb[:], table12[nt * P:(nt + 1) * P, 0:D + H12])
                        nc.sync.dma_start(dbg_t[nt * P:(nt + 1) * P, 0:D + H12], gtb[:])
                    break
                edge_phase(layer)
                if dbg == "h" and layer == n_layers - 1:
                    for nt in range(NBT):
                        nc.sync.dma_start(dbg_t[nt * P:(nt + 1) * P, 0:D],
                                          h_sb[:, nt * D:(nt + 1) * D])
                    break

    nc.compile()
    return nc


def kernel(_trace=False, _trace_kwargs=None, **inputs):
    from concourse.bass_utils import run_bass_kernel_spmd

    x = np.asarray(inputs["x"], np.float32)
    src = np.asarray(inputs["src"]).astype(np.int64)
    dst = np.asarray(inputs["dst"]).astype(np.int64)
    W1 = np.asarray(inputs["W1"], np.float32)
    W2 = np.asarray(inputs["W2"], np.float32)
    W3 = np.asarray(inputs["W3"], np.float32)
    res_W3 = np.asarray(inputs["res_W3"], np.float32)
    al1 = np.asarray(inputs["al1"], np.float32)
    ar1 = np.asarray(inputs["ar1"], np.float32)
    al2 = np.asarray(inputs["al2"], np.float32)
    ar2 = np.asarray(inputs["ar2"], np.float32)
    al3 = np.asarray(inputs["al3"], np.float32)
    ar3 = np.asarray(inputs["ar3"], np.float32)

    def ext(W, al, ar, nh, res=None):
        Wr = W.reshape(W.shape[0], nh, -1)
        wel = np.einsum("khf,hf->kh", Wr, al)
        wer = np.einsum("khf,hf->kh", Wr, ar)
        parts = [W, wel, wer] + ([res] if res is not None else [])
        return np.ascontiguousarray(np.concatenate(parts, axis=1), dtype=np.float32)

    w1e = ext(W1, al1, ar1, H12)                 # [128, 264]
    w2e = ext(W2, al2, ar2, H12)                 # [256, 264]
    w3e = ext(W3, al3, ar3, 1, res_W3)           # [256, 130]

    tile_block, tile_half, TT, idx16, dstpos = make_schedule(src, dst)
    nc = build_nc(tile_block, tile_half, TT)

    iota = np.broadcast_to(np.arange(P, dtype=np.float32)[None, :], (P, P)).copy()
    ident = np.eye(P, dtype=np.float32)

    in_maps = []
    for k in range(NC):
        xk = x[k * NLOC:(k + 1) * NLOC].T                     # [128, 6250]
        xk = np.pad(xk, ((0, 0), (0, NBT * P - NLOC)))
        in_maps.append({
            "xT": np.ascontiguousarray(xk, np.float32),
            "w1": w1e, "w2": w2e, "w3": w3e,
            "idx16": np.ascontiguousarray(idx16[k]),
            "dstpos": np.ascontiguousarray(dstpos[k]),
            "iota": iota, "ident": ident,
        })

    kernel.last_nc = nc
    kernel.last_in_maps = in_maps
    res = run_bass_kernel_spmd(nc, in_maps, core_ids=list(range(NC)),
                               trace=_trace, **(_trace_kwargs or {}))
    out = np.concatenate([res.results[k]["out"] for k in range(NC)], axis=0)
    if _trace:
        kernel.last_result = res
    return out.astype(np.float32)

